# revision 19
# baseline (speedup 1.0000x reference)
"""ConvLSTM cell (B=32, C_IN=32, HC=64, H=W=64, K=3) on 8 trn2 NeuronCores.

Strategy: data-parallel over batch (4 images per core), weights replicated.
The fused conv (-> 256 gate channels) runs as 7 matmuls per 128-channel
chunk per 8-row sub-block (vs 9 shifted-tap matmuls): conv taps are packed
into full-128-partition contraction windows by stacking row-shifted copies
of h/x in the partition dim:
  L1 = [h(dy=0) | h(dy=1)]            -> windows at dx=0,1,2 (col-AP shift)
  L2 = [h(dy=2) | x(dy=0) | x(dy=1)]  -> windows at dx=0,1,2
  W7 = [x(dy=2,dx=0) | x(dy=2,dx=1) | x(dy=2,dx=2)] -> one full-width window
W7's dx shifts are baked on-chip: one DMA (dx=1 block) plus two DVE bf16
column-shifted copies. PE work drops to ~78% of the 9-tap schedule.
Gate chunks: chunk0 = [f, i], chunk1 = [o, g]. Cell-state in/out and all
element-wise math are bf16 (DVE 2x mode); PSUM accumulation stays f32.
Output h_new/c_new is bf16, converted to f32 on host.
"""

import os
import sys

import numpy as np

if "/opt/trn_rl_repo" not in sys.path:
    sys.path.insert(0, "/opt/trn_rl_repo")

import ml_dtypes

BF16 = ml_dtypes.bfloat16

B, C_IN, HC, H, W, K = 32, 32, 64, 64, 64, 3
N_CORES = 8
B_LOC = B // N_CORES  # 4 images per core
SUB_ROWS = 8  # rows per matmul (<= one PSUM bank at 64 cols)
NWIN = 7
# (cout0, ncols, cin0) per dx for the col-AP-shifted windows
COLMAP = {0: (1, 63, 0), 1: (0, 64, 0), 2: (0, 63, 1)}
# per-image block row plans: small lead-in/tail blocks
BLOCK_PLAN = [
    [8, 8, 16, 16, 16],
    [16, 16, 16, 16],
    [16, 16, 16, 16],
    [16, 16, 16, 8, 8],
]

_CACHE: dict = {}


def _build_program():
    import concourse.bacc as bacc
    import concourse.mybir as mybir
    import concourse.tile as tile

    nc = bacc.Bacc("TRN2", target_bir_lowering=False, debug=False)
    f32 = mybir.dt.float32
    bf16 = mybir.dt.bfloat16
    AF = mybir.ActivationFunctionType

    xh_d = nc.dram_tensor("xh", [B_LOC, C_IN + HC, H, W], bf16, kind="ExternalInput").ap()
    c_d = nc.dram_tensor("c", [B_LOC, HC, H, W], bf16, kind="ExternalInput").ap()
    w_d = nc.dram_tensor("w", [128, 2 * NWIN * 128], bf16, kind="ExternalInput").ap()
    b_d = nc.dram_tensor("bias", [128, 2], f32, kind="ExternalInput").ap()
    # out[:, 0] = h_new, out[:, 1] = c_new
    out_d = nc.dram_tensor(
        "out", [B_LOC, 2, HC, H, W], bf16, kind="ExternalOutput"
    ).ap()

    with tile.TileContext(nc) as tc:
        with (
            tc.tile_pool(name="const", bufs=1) as constp,
            tc.tile_pool(name="l1", bufs=3) as l1p,
            tc.tile_pool(name="l2", bufs=3) as l2p,
            tc.tile_pool(name="w7", bufs=3) as w7p,
            tc.tile_pool(name="cin", bufs=3) as cinp,
            tc.tile_pool(name="psum0", bufs=2, space="PSUM") as pp0,
            tc.tile_pool(name="psum1", bufs=2, space="PSUM") as pp1,
            tc.tile_pool(name="work", bufs=3) as sp,
        ):
            w_sb = constp.tile([128, 2 * NWIN * 128], bf16)
            # split per chunk so chunk0 matmuls start before chunk1 lands
            nc.scalar.dma_start(
                w_sb[:, 0 : NWIN * 128], w_d[:, 0 : NWIN * 128]
            )
            nc.scalar.dma_start(
                w_sb[:, NWIN * 128 :], w_d[:, NWIN * 128 :]
            )
            b_sb = constp.tile([128, 2], f32)
            nc.scalar.dma_start(b_sb[:], b_d)

            def stage_b(st):
                # deferred tail of a block: tanh(c_new), h_new, output DMA
                b_, y0_, rpb_, so_, chn_, i_ = st
                tch = sp.tile([64, rpb_ * W], bf16, tag="tch", name=f"tch{i_}")
                nc.scalar.activation(tch[:], chn_[64:128, :], AF.Tanh)
                nc.vector.tensor_mul(chn_[0:64, :], so_[:], tch[:])
                nc.sync.dma_start(
                    out_d[b_, :, :, y0_ : y0_ + rpb_, :].rearrange(
                        "t c y x -> (t c) y x"
                    ),
                    chn_[:].rearrange("p (y x) -> p y x", x=W),
                )

            # PE prewarm: dummy matmuls on zeroed tiles so the HAM clock
            # gate opens before the first real matmul arrives
            dw = constp.tile([128, 128], bf16)
            nc.gpsimd.memset(dw[:], 0.0)
            drh = constp.tile([128, SUB_ROWS * W], bf16)
            nc.gpsimd.memset(drh[:], 0.0)
            pwp = pp0.tile([128, 8 * W], f32, tag="P0", name="pw")
            for _ in range(3):
                nc.tensor.matmul(pwp[:], dw[:], drh[:], start=True, stop=True)

            pending = None
            bi = 0
            for b in range(B_LOC):
                nblk = len(BLOCK_PLAN[b])
                y0 = 0
                for blk, rpb in enumerate(BLOCK_PLAN[b]):
                    top = y0 == 0
                    bot = y0 + rpb == H
                    blk_px = rpb * W

                    # ---- input tiles ----
                    L1 = l1p.tile([128, rpb, W], bf16, tag="L1", name=f"L1_{bi}")
                    # A: h rows y0-1 .. y0+rpb-2  (dy=0)
                    if top:
                        nc.gpsimd.memset(L1[0:64, 0, :], 0.0)
                        nc.sync.dma_start(
                            L1[0:64, 1:rpb, :], xh_d[b, 32:96, 0 : rpb - 1, :]
                        )
                    else:
                        nc.sync.dma_start(
                            L1[0:64, :, :], xh_d[b, 32:96, y0 - 1 : y0 + rpb - 1, :]
                        )
                    # B: h rows y0 .. y0+rpb-1  (dy=1)
                    nc.sync.dma_start(
                        L1[64:128, :, :], xh_d[b, 32:96, y0 : y0 + rpb, :]
                    )

                    L2 = l2p.tile([128, rpb, W], bf16, tag="L2", name=f"L2_{bi}")
                    # h2: h rows y0+1 .. y0+rpb  (dy=2)
                    if bot:
                        nc.gpsimd.memset(L2[0:64, rpb - 1, :], 0.0)
                        nc.sync.dma_start(
                            L2[0:64, 0 : rpb - 1, :],
                            xh_d[b, 32:96, y0 + 1 : y0 + rpb, :],
                        )
                    else:
                        nc.sync.dma_start(
                            L2[0:64, :, :], xh_d[b, 32:96, y0 + 1 : y0 + rpb + 1, :]
                        )
                    # first blocks: spread the x/W7 loads across queues so
                    # the first matmuls aren't gated by gpsimd queue depth
                    xq = nc.gpsimd if bi >= 2 else nc.sync
                    w7q = nc.gpsimd if bi >= 2 else nc.scalar
                    # x0: x rows y0-1 ..  (dy=0)
                    if top:
                        nc.gpsimd.memset(L2[64:96, 0, :], 0.0)
                        xq.dma_start(
                            L2[64:96, 1:rpb, :], xh_d[b, 0:32, 0 : rpb - 1, :]
                        )
                    else:
                        xq.dma_start(
                            L2[64:96, :, :], xh_d[b, 0:32, y0 - 1 : y0 + rpb - 1, :]
                        )
                    # x1: x rows y0 ..  (dy=1)
                    xq.dma_start(
                        L2[96:128, :, :], xh_d[b, 0:32, y0 : y0 + rpb, :]
                    )

                    # W7: [x(2,0) | x(2,1) | x(2,2)]; dx=1 block DMA'd,
                    # dx=0/2 are column-shifted DVE copies of it
                    W7 = w7p.tile([96, rpb, W], bf16, tag="W7", name=f"W7_{bi}")
                    if bot:
                        nc.gpsimd.memset(W7[32:64, rpb - 1, :], 0.0)
                        w7q.dma_start(
                            W7[32:64, 0 : rpb - 1, :],
                            xh_d[b, 0:32, y0 + 1 : y0 + rpb, :],
                        )
                    else:
                        w7q.dma_start(
                            W7[32:64, :, :], xh_d[b, 0:32, y0 + 1 : y0 + rpb + 1, :]
                        )
                    nc.gpsimd.memset(W7[0:32, :, 0:1], 0.0)
                    nc.vector.tensor_copy(W7[0:32, :, 1:W], W7[32:64, :, 0 : W - 1])
                    nc.gpsimd.memset(W7[64:96, :, W - 1 : W], 0.0)
                    nc.vector.tensor_copy(W7[64:96, :, 0 : W - 1], W7[32:64, :, 1:W])

                    cin = cinp.tile([64, blk_px], bf16, tag="cin", name=f"cin{bi}")
                    nc.gpsimd.dma_start(
                        cin[:].rearrange("c (y x) -> c y x", x=W),
                        c_d[b, :, y0 : y0 + rpb, :],
                    )

                    # ---- matmuls: 7 windows per chunk per 8-row sub ----
                    P0 = pp0.tile([128, blk_px], f32, tag="P0", name=f"P0_{bi}")
                    P1 = pp1.tile([128, blk_px], f32, tag="P1", name=f"P1_{bi}")
                    # window list: (tile, K, dx, widx)
                    wins = [
                        (L1, 128, 1, 0),
                        (L1, 128, 0, 1),
                        (L1, 128, 2, 2),
                        (L2, 128, 0, 3),
                        (L2, 128, 1, 4),
                        (L2, 128, 2, 5),
                        (W7, 96, None, 6),
                    ]
                    last = b == B_LOC - 1 and blk == nblk - 1
                    # last block: chunk1 first so so/TG ACTs overlap the
                    # chunk0 matmuls, shortening the serial tail chain
                    chunk_order = ((1, P1), (0, P0)) if last else ((0, P0), (1, P1))
                    for chunk, P in chunk_order:
                        P3 = P[:].rearrange("c (y x) -> c y x", x=W)
                        for wi, (src, kk, dx, widx) in enumerate(wins):
                            lo = (chunk * NWIN + widx) * 128
                            lhsT = w_sb[0:kk, lo : lo + 128]
                            if dx is None:
                                cout0, ncols, cin0 = 0, 64, 0
                            else:
                                cout0, ncols, cin0 = COLMAP[dx]
                            for r0 in range(0, rpb, SUB_ROWS):
                                r1 = min(r0 + SUB_ROWS, rpb)
                                nc.tensor.matmul(
                                    P3[:, r0:r1, cout0 : cout0 + ncols],
                                    lhsT,
                                    src[0:kk, r0:r1, cin0 : cin0 + ncols],
                                    start=(wi == 0),
                                    stop=(wi == NWIN - 1),
                                )

                    # ---- elementwise LSTM math (bf16, DVE 2x) ----
                    # P0 = [f | i], P1 = [o | g] (by 64-partition halves)
                    s_fi = sp.tile([128, blk_px], bf16, tag="sfi", name=f"sfi{bi}")
                    so = sp.tile([64, blk_px], bf16, tag="so", name=f"so{bi}")
                    TG = sp.tile([128, blk_px], bf16, tag="tg", name=f"tg{bi}")

                    def act_fi():
                        nc.scalar.activation(
                            s_fi[:], P0[:], AF.Sigmoid, bias=b_sb[:, 0:1]
                        )

                    def act_og():
                        nc.scalar.activation(
                            so[:], P1[0:64, :], AF.Sigmoid, bias=b_sb[0:64, 1:2]
                        )
                        nc.scalar.activation(
                            TG[64:128, :],
                            P1[64:128, :],
                            AF.Tanh,
                            bias=b_sb[64:128, 1:2],
                        )

                    if last:
                        act_og()
                        act_fi()
                    else:
                        act_fi()
                        act_og()
                    # tA = f*c, tB = i*g, c_new = tA + tB
                    tA = sp.tile([64, blk_px], bf16, tag="tA", name=f"tA{bi}")
                    nc.vector.tensor_mul(tA[:], s_fi[0:64, :], cin[:])
                    tB = sp.tile([64, blk_px], bf16, tag="tB", name=f"tB{bi}")
                    nc.vector.tensor_mul(tB[:], s_fi[64:128, :], TG[64:128, :])
                    # chn = [h_new | c_new] merged output tile
                    chn = sp.tile([128, blk_px], bf16, tag="chn", name=f"chn{bi}")
                    nc.vector.tensor_add(chn[64:128, :], tA[:], tB[:])
                    if pending is not None:
                        stage_b(pending)
                    pending = (b, y0, rpb, so, chn, bi)
                    bi += 1
                    y0 += rpb
            stage_b(pending)

    nc.compile()
    return nc


def get_program():
    if "nc" not in _CACHE:
        _CACHE["nc"] = _build_program()
    return _CACHE["nc"]


def _prep_host(inputs):
    """Pack weights/biases; convert x/h/c to bf16; build per-core input maps."""
    x = np.asarray(inputs["x"], np.float32)
    h = np.asarray(inputs["hidden_state"], np.float32)
    c = np.ascontiguousarray(np.asarray(inputs["cell_state"], np.float32)).astype(BF16)

    # gate order [f, i] (chunk0), [o, g] (chunk1)
    gx = [inputs["w_xf"], inputs["w_xi"], inputs["w_xo"], inputs["w_xg"]]
    gh = [inputs["w_hf"], inputs["w_hi"], inputs["w_ho"], inputs["w_hg"]]
    wx = np.stack([np.asarray(a, np.float32) for a in gx])  # [4, HC, C_IN, 3, 3]
    wh = np.stack([np.asarray(a, np.float32) for a in gh])  # [4, HC, HC, 3, 3]

    def wxT(chunk, dy, dx):  # -> [C_IN, 128]
        blk = wx[2 * chunk : 2 * chunk + 2, :, :, dy, dx]  # [2, HC, C_IN]
        return np.transpose(blk, (2, 0, 1)).reshape(C_IN, 2 * HC)

    def whT(chunk, dy, dx):  # -> [HC, 128]
        blk = wh[2 * chunk : 2 * chunk + 2, :, :, dy, dx]
        return np.transpose(blk, (2, 0, 1)).reshape(HC, 2 * HC)

    wpack = np.zeros((128, 2 * NWIN * 128), np.float32)
    for chunk in range(2):
        wlist = [
            np.concatenate([whT(chunk, 0, 1), whT(chunk, 1, 1)], 0),  # W1(1)
            np.concatenate([whT(chunk, 0, 0), whT(chunk, 1, 0)], 0),  # W1(0)
            np.concatenate([whT(chunk, 0, 2), whT(chunk, 1, 2)], 0),  # W1(2)
            np.concatenate(
                [whT(chunk, 2, 0), wxT(chunk, 0, 0), wxT(chunk, 1, 0)], 0
            ),  # W2(0)
            np.concatenate(
                [whT(chunk, 2, 1), wxT(chunk, 0, 1), wxT(chunk, 1, 1)], 0
            ),  # W2(1)
            np.concatenate(
                [whT(chunk, 2, 2), wxT(chunk, 0, 2), wxT(chunk, 1, 2)], 0
            ),  # W2(2)
            np.concatenate(
                [wxT(chunk, 2, 0), wxT(chunk, 2, 1), wxT(chunk, 2, 2)], 0
            ),  # W7 (96 rows)
        ]
        for widx, wmat in enumerate(wlist):
            lo = (chunk * NWIN + widx) * 128
            wpack[0 : wmat.shape[0], lo : lo + 128] = wmat
    wpack = wpack.astype(BF16)

    bf = np.asarray(inputs["b_xf"], np.float32) + np.asarray(inputs["b_hf"], np.float32)
    bi_ = np.asarray(inputs["b_xi"], np.float32) + np.asarray(inputs["b_hi"], np.float32)
    bo = np.asarray(inputs["b_xo"], np.float32) + np.asarray(inputs["b_ho"], np.float32)
    bg = np.asarray(inputs["b_xg"], np.float32) + np.asarray(inputs["b_hg"], np.float32)
    bias = np.stack(
        [np.concatenate([bf, bi_]), np.concatenate([bo, bg])], axis=1
    ).astype(np.float32)  # [128, 2]

    xh = np.concatenate([x, h], axis=1).astype(BF16)  # [B, 96, H, W]

    in_maps = []
    for i in range(N_CORES):
        s = slice(i * B_LOC, (i + 1) * B_LOC)
        in_maps.append(
            {
                "xh": xh[s],
                "c": c[s],
                "w": wpack,
                "bias": bias,
            }
        )
    return in_maps


def run(inputs, trace=False, trace_kwargs=None):
    from concourse.bass_utils import run_bass_kernel_spmd

    nc = get_program()
    in_maps = _prep_host(inputs)
    res = run_bass_kernel_spmd(
        nc,
        in_maps,
        list(range(N_CORES)),
        trace=trace,
        **(trace_kwargs or {}),
    )
    h_new = np.concatenate([r["out"][:, 0] for r in res.results], 0).astype(
        np.float32
    )
    c_new = np.concatenate([r["out"][:, 1] for r in res.results], 0).astype(
        np.float32
    )
    return (h_new, c_new), res


def kernel(**inputs):
    (h_new, c_new), _ = run(inputs, trace=False)
    return (h_new, c_new)
